# revision 31
# baseline (speedup 1.0000x reference)
"""Trainium2 Bass kernel for nn_BasicTT (TT-decomposed 3-layer MLP + log_softmax).

Strategy (8-way batch data parallelism, b=256 per core):
  Host prep (numpy):
    - Merge layer-1 TT cores 3,4,5 -> gA [128=(k-tiles), 512]
    - Merge layer-1 cores 1,2 (+ layer-1 bias in pad rows) -> gB [128, 64]
    - Layer 2 and 3 TT weights densified: g2 [128, (v16, 64)], g3 [64, 32]
    - Final linear reduced to the logit difference D (incl. the logit
      bias via a constant-1 h3 row); log_softmax = -softplus([D, -D])
    - x pre-transposed per core to xT [128, (k4, bank2, b8, j24, s2)]:
      the (j, s) interleave lets ONE strided copy stage each chunk's two
      psum banks into the fp16-pair transpose buffer
  Device (per core, fp16 matmuls), tensor-engine-bound body (~91% busy):
    - DMA order: gA..dlt16 in one 128-descriptor DMA, then x chunks on
      gpsimd triggers (chunks 5-7 WAR-gated by the 5-buffer x pool); g2
      trigger delayed into the loop so its descriptors trail x0-x3
      (queues are descriptor-rate-bound, ~200ns/desc regardless of size);
      12 warm-up matmuls ramp the PE clock p-states during the DMA wait
    - per chunk (b32): 8 accumulating K=128 A-matmuls into one 2-bank
      psum tile, bank1 with reversed k-order so no LDWEIGHTS exposure at
      the bank boundary; ONE copy (Scalar/DVE by parity) stages psum ->
      sg fp16 (pads hold bias deltas); DVE 32x32 stream-transpose on the
      int32 view; 2 phase-B matmuls; relu+split per m3l (Scalar + DVE)
      -> h1 [128, (v16, b256)] (h1 column stride MUST stay 1 for L2's
      rhs: strided matmul rhs is ~3x slower on real HW)
    - software pipeline: B/splits of chunk c emitted after A of chunk
      c+1 so the tensor queue never waits on the DVE transpose; A and B
      psum tiles share one 3-slot pool (WAR deps land ~2.5 chunks back)
    - L2/L3/logit-diff/softplus tail: batch-half 0 spread over
      iterations 4-6, quarter 2 at iteration 7, quarter 3 post-loop
      (short drain); softplus(D) = relu(D) + ln(1+exp(-|D|)) with the
      elementwise chain on Scalar (Abs/Exp(scale=-1)/Ln), fuse on DVE
"""
import os
import numpy as np

NCORES = 8
B = 2048
BLOC = B // NCORES  # 256
NCH = BLOC // 32    # 8 chunks of 32 samples

_prog_cache = {}


# ---------------------------------------------------------------------------
# Host-side weight preparation
# ---------------------------------------------------------------------------
def _tt_full_matrix(cores):
    """Dense matrix W [prod(m), prod(n)] of a TT layer, matching the
    reference tt_linear index convention."""
    n = 1
    for G in cores:
        n *= G.shape[2]
    x = np.eye(n)
    b = n
    z = x.reshape(b, 1, -1)
    for G in cores:
        r0, m, nn_, r1 = G.shape
        z4 = z.reshape(b, r0, nn_, -1)
        z = np.einsum('brns,rmnq->bqsm', z4, G).reshape(b, r1, -1)
    return z.reshape(b, -1).T


def _build_host_tensors(p):
    f64 = {k: np.asarray(v, np.float64) for k, v in p.items()}

    # G345 = l1c2 (r2,m3,n3,r3) * l1c3 (r3,m4,n4,r4) * l1c4 (r4,m5,n5,1)
    g34 = np.einsum('amcb,bndq->amncdq', f64['l1c2'], f64['l1c3'])
    g345 = np.einsum('amncdq,qpe->amnpcde', g34, f64['l1c4'][:, :, :, 0])
    # g345[r2,m3,m4,m5,n3,n4,n5] -> lhsT_A [(n3,n4,n5)=512, (r2,m3,m4,m5)=128]
    lhsT_A = g345.transpose(4, 5, 6, 0, 1, 2, 3).reshape(512, 128)
    # pack K-tiles side by side: gA[c128, k*128 + p] = lhsT_A[k*128+c, p]
    gA = np.ascontiguousarray(
        lhsT_A.reshape(4, 128, 128).transpose(1, 0, 2).reshape(128, 512))

    # G12 = l1c0 (1,m1,n1,r1) * l1c1 (r1,m2,n2,r2) -> g12[n1,n2,r2,m1,m2]
    g12 = np.einsum('mar,rnbq->abqmn', f64['l1c0'][0], f64['l1c1'])
    b1 = f64['b1']  # (m1,m2,m3,m4,m5) = (8,4,4,4,4)
    lhsT_B = np.zeros((128, 64))
    for r2 in range(2):
        for m3h in range(2):
            g = r2 * 2 + m3h
            for n1 in range(3):
                for n2 in range(8):
                    j = n1 * 8 + n2
                    for m1 in range(8):
                        for m2 in range(4):
                            lhsT_B[g * 32 + j, m3h * 32 + m1 * 4 + m2] = \
                                g12[n1, n2, r2, m1, m2]
    # bias rows: row (g, 24+j') fires for u = 8g+j' (delta pattern sits in
    # the sg staging-buffer pad cols, written once at startup)
    for g in range(4):
        for jp in range(8):
            u = 8 * g + jp
            m3l, m4, m5 = u >> 4, (u >> 2) & 3, u & 3
            for m3h in range(2):
                for m1 in range(8):
                    for m2 in range(4):
                        m3 = m3h * 2 + m3l
                        lhsT_B[g * 32 + 24 + jp, m3h * 32 + m1 * 4 + m2] = \
                            b1[m1, m2, m3, m4, m5]
    # delta pattern for the sg pads, layout [128, (b'16, j'8, s2)]:
    # row p=(g,u) has 1.0 at pad col j' iff u == 8g+j', same for both s
    dlt = np.zeros((128, 8))
    for gg in range(4):
        for u in range(32):
            jp = u - 8 * gg
            if 0 <= jp < 8:
                dlt[gg * 32 + u, jp] = 1.0
    dltrep = np.zeros((128, 16, 8, 2))
    dltrep[:, :, :, 0] = dlt[:, None, :]
    dltrep[:, :, :, 1] = dlt[:, None, :]
    dltrep = dltrep.reshape(128, 256)

    # dense layer 2/3
    W2 = _tt_full_matrix([f64['l2c0'], f64['l2c1'], f64['l2c2'],
                          f64['l2c3'], f64['l2c4']])  # [64, 2048]
    W3 = _tt_full_matrix([f64['l3c0'], f64['l3c1'], f64['l3c2'],
                          f64['l3c3'], f64['l3c4']])  # [32, 64]
    # g2 [128=(m3l,m3h,m1,m2), (v16, o64)]
    g2 = np.zeros((128, 16, 64))
    for pp in range(128):
        m3l, m3h = pp >> 6, (pp >> 5) & 1
        m1, m2 = (pp >> 2) & 7, pp & 3
        m3 = m3h * 2 + m3l
        for v in range(16):
            m4, m5 = v >> 2, v & 3
            flat = (((m1 * 4 + m2) * 4 + m3) * 4 + m4) * 4 + m5
            g2[pp, v, :] = W2[:, flat]
    g2 = g2.reshape(128, 1024)

    # wd gets a 33rd row carrying the logit-bias difference; the device
    # h3 tile has a constant-1.0 row 32 so the d-matmul adds it for free
    wd = np.zeros((33, 2))
    wd[0:32, 0] = f64['W'][1] - f64['W'][0]
    wd[0:32, 1] = -(f64['W'][1] - f64['W'][0])
    wd[32, 0] = f64['bl'][1] - f64['bl'][0]
    wd[32, 1] = f64['bl'][0] - f64['bl'][1]

    # fp16 consts, two DMAs (early | late):
    #   early: gA 0:512 | gB 512:576 | g3 576:608 (rows 0:64)
    #          | wd 608:610 (rows 0:33) | dlt16 610:866
    #   late:  g2 866:1890 | b2 1890:1892 | b3 1892:1894 (f32 bit pairs)
    cstH = np.zeros((128, 1894), np.float16)
    cstH[:, 0:512] = gA.astype(np.float16)
    cstH[:, 512:576] = lhsT_B.astype(np.float16)
    cstH[0:64, 576:608] = W3.T.astype(np.float16)
    cstH[0:33, 608:610] = wd.astype(np.float16)
    cstH[:, 610:866] = dltrep.astype(np.float16)
    cstH[:, 866:1890] = g2.astype(np.float16)
    cstH[0:64, 1890:1892] = np.ascontiguousarray(
        f64['b2'].ravel().astype(np.float32)[:, None]).view(np.float16)
    cstH[0:32, 1892:1894] = np.ascontiguousarray(
        f64['b3'].ravel().astype(np.float32)[:, None]).view(np.float16)
    return dict(cstH=cstH)


def _make_xT(x_core16):
    # x_core16: fp16 [256, 12288] with flat = j*512 + k*128 + p
    # -> [128p*(8ch), (k4, bank2, b8, j24, s2)]; sample within chunk =
    # bank*16 + b*2 + s so fp16 batch PAIRS (s) are adjacent in psum cols
    xr = x_core16.reshape(8, 2, 8, 2, 24, 4, 128)  # ch,bank,b8,s,j,k,p
    return np.ascontiguousarray(
        xr.transpose(0, 6, 5, 1, 2, 4, 3)).reshape(1024, 3072)


# ---------------------------------------------------------------------------
# Device program
# ---------------------------------------------------------------------------
def _patch_act_tables():
    """Restrict the activation-table chooser to the one table that holds
    every function this kernel uses (Copy/Relu/Exp/Ln), so exactly one
    table load is emitted."""
    import concourse.hw_specs as hw_specs
    import concourse.bacc as bacc_mod
    if getattr(bacc_mod, '_att_patched', False):
        return
    orig = hw_specs.get_activation_tables

    def patched(arch):
        t = orig(arch)
        keep = 'natural_log_exp_and_others'
        if keep not in t:
            return t
        return {name: (s if name == keep else set())
                for name, s in t.items()}

    bacc_mod.get_activation_tables = patched
    bacc_mod._att_patched = True


def _build_program():
    if 'nc' in _prog_cache:
        return _prog_cache['nc']
    from contextlib import ExitStack
    import concourse.bacc as bacc
    import concourse.mybir as mybir
    import concourse.tile as tile

    _patch_act_tables()

    F16 = mybir.dt.float16
    F32 = mybir.dt.float32
    I32 = mybir.dt.int32
    AF = mybir.ActivationFunctionType

    nc = bacc.Bacc(None, target_bir_lowering=False)

    xT = nc.declare_dram_parameter("xT", [1024, 3072], F16, isOutput=False)
    cstH = nc.declare_dram_parameter("cstH", [128, 1894], F16, isOutput=False)
    y = nc.declare_dram_parameter("y", [2, BLOC], F32, isOutput=True)

    with tile.TileContext(nc) as tc, ExitStack() as ctx:
        consts = ctx.enter_context(tc.tile_pool(name="consts", bufs=1))
        xpool = ctx.enter_context(tc.tile_pool(name="x", bufs=5))
        sgpool = ctx.enter_context(tc.tile_pool(name="sg", bufs=1))
        tbpool = ctx.enter_context(tc.tile_pool(name="tb", bufs=3))
        h1pool = ctx.enter_context(tc.tile_pool(name="h1", bufs=1))
        spool = ctx.enter_context(tc.tile_pool(name="small", bufs=1))
        # one shared 3-slot pool for the A (2-bank) and B (2-bank) psum
        # tiles: the alternating A/B allocation order gives every tile a
        # WAR dependency ~2.5 chunks back, so nothing serializes
        psAB = ctx.enter_context(tc.tile_pool(name="psAB", bufs=3,
                                              space="PSUM"))
        psT = ctx.enter_context(tc.tile_pool(name="psT", bufs=1, space="PSUM"))

        # ---- DMA issue order: gA + small consts first (scalar trigger),
        # x chunks on gpsimd (4-7 WAR-gated by the 4-buffer pool), g2
        # triggered from scalar AFTER the act warm-up so its descriptors
        # land behind x chunk 0/1 in the queues.
        cH = consts.tile([128, 1894], F16, tag="cstH")
        nc.scalar.dma_start(cH[:, 0:866], cstH[:, 0:866])

        # warm-up memsets go before the x DMA triggers on gpsimd so the
        # PE ramp matmuls can start immediately
        scr = spool.tile([2, 32], F32, tag="scr")
        nc.gpsimd.memset(scr[:, :], 0)
        wsc = spool.tile([128, 640], F16, tag="wsc")
        nc.gpsimd.memset(wsc[:, :], 0)

        xts = []
        for bc in range(NCH):
            xt = xpool.tile([128, 3072], F16, tag="xt", name=f"xt_{bc}")
            xts.append(xt)
        for bc in range(5):
            nc.gpsimd.dma_start(xts[bc][:, :], xT[bc * 128:(bc + 1) * 128, :])

        gA_t = cH[:, 0:512]
        gB_t = cH[:, 512:576]
        g3_t = cH[0:64, 576:608]
        wd_t = cH[0:33, 608:610]
        dlt16_t = cH[:, 610:866]
        g2_t = cH[:, 866:1890]
        b2_t = cH[0:64, 1890:1892].bitcast(F32)
        b3_t = cH[0:32, 1892:1894].bitcast(F32)

        # ---- warm-up: act-table load + PE clock ramp during the DMA wait
        scr2 = spool.tile([2, 32], F32, tag="scr2")
        nc.scalar.activation(scr2[:, :], scr[:, :], AF.Copy)
        nc.scalar.activation(scr2[:, :], scr[:, :], AF.Relu)
        nc.scalar.activation(scr2[:, :], scr[:, :], AF.Exp)
        nc.scalar.activation(scr2[:, :], scr[:, :], AF.Ln, bias=1.0)
        paw = psAB.tile([128, 1024], F32, tag="psAB", name="psAB_warm")
        for w in range(12):
            nc.tensor.matmul(paw[:, (w % 2) * 512:(w % 2) * 512 + 512],
                             wsc[:, 0:128], wsc[:, 128:640],
                             start=True, stop=True)

        h1 = h1pool.tile([128, BLOC * 16], F16)
        h1v = h1.rearrange("p (v b) -> p v b", b=BLOC)

        # persistent staging buffers: pad cols (e=48..63 per b-block) hold
        # the bias-delta pattern, written once; staging never touches them
        sg_bufs = []
        for i in range(3):
            z = sgpool.tile([128, 1024], F16, tag=f"sg{i}")
            nc.vector.tensor_copy(
                z.rearrange("p (b e) -> p b e", e=64)[:, :, 48:64],
                dlt16_t.rearrange("p (b e) -> p b e", e=16))
            sg_bufs.append(z)

        # tail psum: p2 [64, 0:256] | p3 [32, 256:512] | pd [2, 512:768]
        pT = psT.tile([64, 768], F32, tag="pT")
        p2 = pT[:, 0:256]
        h2 = spool.tile([64, BLOC], F16, tag="h2")
        # h3 row 32 is a constant 1.0 so the d-matmul adds the logit-bias
        # difference (wd row 32) for free
        h3 = spool.tile([33, BLOC], F16, tag="h3")
        nc.gpsimd.memset(h3[32:33, :], 1.0)
        na = spool.tile([2, BLOC], F32, tag="na")
        ex = spool.tile([2, BLOC], F32, tag="ex")
        ln1 = spool.tile([2, BLOC], F32, tag="ln1")
        rl = spool.tile([2, BLOC], F32, tag="rl")
        out = spool.tile([2, BLOC], F32, tag="out")

        def l2_mm(c0, c1):
            # batch-col range [c0, c1): depends on those h1 chunks
            for v in range(16):
                nc.tensor.matmul(
                    p2[:, c0:c1],
                    g2_t[:, v * 64:(v + 1) * 64],
                    h1v[:, v, c0:c1],
                    start=(v == 0), stop=(v == 15))

        def tail_mm(c0, c1):
            # relu(+b2) -> L3 -> relu(+b3) -> logit-diff (D incl bias)
            c = slice(c0, c1)
            cp = slice(256 + c0, 256 + c1)
            nc.scalar.activation(h2[:, c], p2[:, c], AF.Relu,
                                 bias=b2_t[:, 0:1])
            nc.tensor.matmul(pT[0:32, cp], g3_t[:, :], h2[:, c],
                             start=True, stop=True)
            nc.scalar.activation(h3[0:32, c], pT[0:32, cp], AF.Relu,
                                 bias=b3_t[:, 0:1])
            nc.tensor.matmul(pT[0:2, slice(512 + c0, 512 + c1)],
                             wd_t[:, :], h3[:, c],
                             start=True, stop=True)

        def tail_sm(c0, c1):
            # y = -softplus(D) = -relu(D) - ln(1 + exp(-|D|)); elementwise
            # chain on Scalar, final fuse + DMA
            c = slice(c0, c1)
            pd = pT[0:2, slice(512 + c0, 512 + c1)]
            nc.scalar.activation(rl[:, c], pd, AF.Relu)
            nc.scalar.activation(na[:, c], pd, AF.Abs)
            nc.scalar.activation(ex[:, c], na[:, c], AF.Exp, scale=-1.0)
            nc.scalar.activation(ln1[:, c], ex[:, c], AF.Ln, bias=1.0)
            nc.vector.scalar_tensor_tensor(out[:, c], ln1[:, c], -1.0,
                                           rl[:, c],
                                           op0=mybir.AluOpType.mult,
                                           op1=mybir.AluOpType.subtract)
            nc.sync.dma_start(y[:, c], out[:, c])

        def stage_a(bc):
            # 8 accumulating A matmuls into a 2-bank psum tile
            xt = xts[bc]
            pa = psAB.tile([128, 1024], F32, tag="psAB", name=f"psA_{bc}")
            for bank in range(2):  # bank-major keeps LDWEIGHTS hidden
                ks = range(4) if bank == 0 else range(3, -1, -1)
                for i, k in enumerate(ks):
                    # bank1 runs k in reverse so its first matmul reuses
                    # the weights bank0 finished with (no LDW exposure)
                    nc.tensor.matmul(
                        pa[:, bank * 512:bank * 512 + 384],
                        gA_t[:, k * 128:(k + 1) * 128],
                        xt[:, k * 768 + bank * 384: k * 768 + (bank + 1) * 384],
                        start=(i == 0), stop=(i == 3))
            return pa

        def stage_cast(bc, pa):
            # stage both psum banks -> sg fp16 in ONE copy; alternate the
            # engine by parity to balance Scalar/Vector load.  The last
            # chunk is on the drain critical path: do its two banks on
            # Scalar + Vector in parallel instead.
            sg = sg_bufs[bc % 3]
            src = pa.rearrange("p (bk c) -> p bk c", bk=2)[:, :, 0:384] \
                .rearrange("p bk (b e) -> p bk b e", e=48)
            dst = sg.rearrange("p (bk b e) -> p bk b e", bk=2, e=64)[
                :, :, :, 0:48]
            if bc == NCH - 1:
                nc.scalar.activation(dst[:, 0:1], src[:, 0:1], AF.Copy)
                nc.vector.tensor_copy(dst[:, 1:2], src[:, 1:2])
            elif bc % 2 == 0:
                nc.vector.tensor_copy(dst, src)
            else:
                nc.scalar.activation(dst, src, AF.Copy)
            tb = tbpool.tile([128, 1024], F16, tag="tb")
            nc.vector.transpose(tb[:, :].bitcast(I32), sg[:, :].bitcast(I32))
            return tb

        def stage_b(bc, tb):
            pb = psAB.tile([64, 1024], F32, tag="psAB", name=f"psB_{bc}")
            for bank in range(2):
                nc.tensor.matmul(pb[:, bank * 512:(bank + 1) * 512],
                                 gB_t[:, :],
                                 tb[:, bank * 512:(bank + 1) * 512],
                                 start=True, stop=True)
            return pb

        def stage_split(bc, pb):
            # relu + split into h1 [(m3l,m3h,m1,m2), (v16, b)]: one op per
            # m3l (both banks at once), Scalar + Vector in parallel
            pb4 = pb.rearrange("p (bb u s) -> p bb u s", u=32, s=2)
            for m3l in range(2):
                src = pb4[:, :, m3l * 16:(m3l + 1) * 16, :]
                dst = h1v[m3l * 64:(m3l + 1) * 64, :,
                          bc * 32:(bc + 1) * 32] \
                    .rearrange("p v (bb s) -> p bb v s", s=2)
                if m3l == 0:
                    nc.scalar.activation(dst, src, AF.Relu)
                else:
                    nc.vector.tensor_scalar_max(dst, src, 0.0)

        # software pipeline: B/splits for chunk c are emitted after A of
        # chunk c+1 so the tensor queue never waits on the DVE transpose
        tbs = {}
        pbs = {}
        for bc in range(NCH):
            if 1 <= bc <= 3:
                # stagger the gated chunk-4..7 triggers so the gpsimd
                # queue releases each as soon as its WAR clears
                nc.gpsimd.dma_start(xts[bc + 4][:, :],
                                    xT[(bc + 4) * 128:(bc + 5) * 128, :])
            pa = stage_a(bc)
            if bc >= 1:
                pbs[bc - 1] = stage_b(bc - 1, tbs.pop(bc - 1))
            tbs[bc] = stage_cast(bc, pa)
            if bc >= 1:
                stage_split(bc - 1, pbs.pop(bc - 1))
            if bc == 1:
                # g2 + bias consts: triggered only now so their queue
                # descriptors trail x chunks 0-3
                nc.scalar.dma_start(cH[:, 866:1894], cstH[:, 866:1894])
            # batch-half 0 (chunks 0-3) spread over iterations 4-6;
            # quarter 2 at iteration 7; quarter 3 post-loop (short drain)
            if bc == 4:
                l2_mm(0, 128)
            elif bc == 5:
                tail_mm(0, 128)
            elif bc == 6:
                tail_sm(0, 128)
            elif bc == 7:
                l2_mm(128, 192)
                tail_mm(128, 192)
                tail_sm(128, 192)
        pbs[7] = stage_b(7, tbs.pop(7))
        stage_split(7, pbs.pop(7))
        l2_mm(192, 256)
        tail_mm(192, 256)
        tail_sm(192, 256)

    nc.compile()
    _prog_cache['nc'] = nc
    return nc


# ---------------------------------------------------------------------------
# Entry point
# ---------------------------------------------------------------------------
def kernel(**inputs):
    from concourse.bass_utils import run_bass_kernel_spmd

    H = _build_host_tensors(inputs)
    x16 = np.asarray(inputs['x'], np.float32).astype(np.float16) \
        .reshape(B, 12288)
    nc = _build_program()

    in_maps = []
    for c in range(NCORES):
        m = dict(H)
        m['xT'] = _make_xT(x16[c * BLOC:(c + 1) * BLOC])
        in_maps.append(m)

    trace = bool(os.environ.get('KERNEL_TRACE'))
    tmpdir = None
    if trace:
        tmpdir = os.environ.get('KERNEL_TRACE_DIR') or None
        if tmpdir:
            os.makedirs(tmpdir, exist_ok=True)
    res = run_bass_kernel_spmd(nc, in_maps, list(range(NCORES)),
                               trace=trace, tmpdir=tmpdir)
    kernel.last_results = res
    out = np.concatenate(
        [res.results[c]['y'].T for c in range(NCORES)], axis=0)
    return np.ascontiguousarray(out, np.float32)


if __name__ == '__main__':
    # smoke test with random inputs shaped per spec
    rng = np.random.default_rng(0)
    shapes = {
        'x': (B, 3, 8, 8, 8, 8),
        'l1c0': (1, 8, 3, 3), 'l1c1': (3, 4, 8, 2), 'l1c2': (2, 4, 8, 2),
        'l1c3': (2, 4, 8, 2), 'l1c4': (2, 4, 8, 1), 'b1': (8, 4, 4, 4, 4),
        'l2c0': (1, 4, 8, 2), 'l2c1': (2, 2, 4, 2), 'l2c2': (2, 2, 4, 2),
        'l2c3': (2, 2, 4, 2), 'l2c4': (2, 2, 4, 1), 'b2': (4, 2, 2, 2, 2),
        'l3c0': (1, 2, 4, 2), 'l3c1': (2, 2, 2, 2), 'l3c2': (2, 2, 2, 2),
        'l3c3': (2, 2, 2, 2), 'l3c4': (2, 2, 2, 1), 'b3': (2, 2, 2, 2, 2),
        'W': (2, 32), 'bl': (2,),
    }
    ins = {k: rng.standard_normal(v).astype(np.float32) * 0.3
           for k, v in shapes.items()}
    print(kernel(**ins)[:4])
